# revision 1
# baseline (speedup 1.0000x reference)
"""Demodulated 3x3 convolution Trainium2 kernel — q-interleaved scheme.

Data-parallel over batch: 16 samples -> 8 cores x 2 samples.

Layout: HBM NHWC rows are fetched with partition = w-PAIR (wpair j covers
w=2j,2j+1) so every DMA descriptor moves 512B contiguous (full DMA rate,
no <512B latency penalty); the fetch is a Pool-engine (SWDGE) DMA that
also casts fp32->bf16 in flight.  PE-transpose turns each row into a
channel-major q-interleaved slab [(q,c) 128 parts, 128 wpair cols] bf16,
ACT-copied into a ring of NR+1 130-col slots (1 halo col each side, +1
pad slot so 2-row windows never wrap).

Conv: for out row-pair Q_j (rows 2j,2j+1) accumulate 9 bf16 matmuls
(3 ky-groups x 3 shifts d) of N=259 against 2-slot ring windows with
block-structured weights lhsT[(q_in,c),(q_out,f)]; block (qi,qo) of
(g,d) holds styled tap W[g, kx], kx = qi - qo + 2d + 1 when 0<=kx<=2.

Epilogue: DVE does E = Q*demod[f] + bias (per-partition operands, bf16
out), PE transposes E back to [wpair, (q,f)], DVE copies PSUM->SBUF
fp32, and the SP output DMA writes 512B descriptors ([s,row,w,f] with
(q,f) inner).

Engine budget per 2-row iteration (cost model): PE 2843 cyc = 1185 ns
(9 conv mms + 4 transposes), DVE ~880, ACT ~610, Pool ~585 (in-DMA),
HWDGE ~625 (out-DMA), DMA engines ~730.
"""

import math
import os
import sys

os.environ["BASS_NEVER_TRACE"] = "1"

for _p in ("/opt/trn_rl_repo",):
    if _p not in sys.path:
        sys.path.insert(0, _p)

import numpy as np

import concourse.bass as bass
import concourse.bacc as bacc
import concourse.mybir as mybir
import concourse.tile as tile
from concourse.bass_utils import run_bass_kernel_spmd

B, H, W, CIN = 16, 256, 256, 64
LATENT, F, KK = 512, 64, 3
NCORES = 8
BS = B // NCORES  # samples per core
J = W // 2  # 128 wpairs
NR = 16  # ring slots (+1 pad slot); must divide 256
RW = (NR + 1) * 130 + 1  # ring cols: 1 zero pad col + (NR+1) slots

F32 = mybir.dt.float32
BF16 = mybir.dt.bfloat16
AF = mybir.ActivationFunctionType
ALU = mybir.AluOpType

# (g, d) -> list of (q_in, q_out, kx)
_BLOCKS = {}
for _g in range(3):
    for _d in (-1, 0, 1):
        bl = []
        for _qi in range(2):
            for _qo in range(2):
                _kx = _qi - _qo + 2 * _d + 1
                if 0 <= _kx <= 2:
                    bl.append((_qi, _qo, _kx))
        _BLOCKS[(_g, _d)] = bl

_CACHE = {}


def _slot(m, s=0):
    """ring slot of image row m for sample s; the +s offset makes sample
    s's row -1 land on the slot sample s-1's row-256 zero-write left
    zeroed, so no re-zero is needed at the sample switch."""
    return (m + 1 + s) % NR


def _scol(s):
    """first col of slot s (its halo-L); j-cols at +1..+128, halo-R at +129."""
    return 1 + s * 130


def _build_nc():
    nc = bacc.Bacc("TRN2", target_bir_lowering=False, debug=False)

    x_h = nc.dram_tensor("x", [BS, H, W, CIN], F32, kind="ExternalInput")
    lat_h = nc.dram_tensor("lat", [BS, LATENT], F32, kind="ExternalInput")
    dw_h = nc.dram_tensor("dw", [LATENT, CIN], F32, kind="ExternalInput")
    db_h = nc.dram_tensor("db", [CIN], F32, kind="ExternalInput")
    ck_h = nc.dram_tensor("ck", [KK, KK, CIN, F], F32, kind="ExternalInput")
    bi_h = nc.dram_tensor("bi", [F], F32, kind="ExternalInput")
    id_h = nc.dram_tensor("ident", [128, 128], F32, kind="ExternalInput")
    out_h = nc.dram_tensor("out", [BS, H, W, F], F32, kind="ExternalOutput")

    const_k = math.sqrt(2.0) / math.sqrt(KK * KK * CIN)
    inv_sqrt_lat = 1.0 / math.sqrt(LATENT)

    with tile.TileContext(nc) as tc:
        with (
            tc.tile_pool(name="const", bufs=1) as cpool,
            tc.tile_pool(name="wpool", bufs=1) as wpool,
        ):
            # --- loads: chunk0+ksty0 on Pool; ident/lat/dw on SP ---
            identD = cpool.tile([128, 128], F32)
            nc.sync.dma_start(identD[:], id_h[:])
            latR = cpool.tile([BS, LATENT], F32)
            nc.sync.dma_start(latR[:], lat_h[:])
            dwD = cpool.tile([128, 4, CIN], F32)
            nc.sync.dma_start(
                dwD[:], dw_h[:].rearrange("(j p) f -> p j f", p=128)
            )
            ksty0 = cpool.tile([64, 9, F], F32)

            # DVE: matmul operand copies first so style matmuls start early
            dwT = cpool.tile([128, 4, CIN], F32)
            nc.vector.tensor_copy(dwT[:], dwD[:])
            identB = cpool.tile([128, 128], BF16)
            nc.vector.tensor_copy(identB[:], identD[:])
            ones64 = cpool.tile([64, 1], F32)
            nc.vector.memset(ones64[:], 1.0)
            zero64 = cpool.tile([64, 1], F32)
            nc.vector.memset(zero64[:], 0.0)
            scrS = cpool.tile([64, 1], F32)
            nc.scalar.activation(scrS[:], zero64[:], AF.Sqrt, bias=zero64[:])
            epsT = cpool.tile([64, F], F32)
            nc.vector.memset(epsT[:], 1e-8 / 64.0)

            zeroB = cpool.tile([128, 128], BF16)
            ring = cpool.tile([128, RW], BF16)

            demodQ = []
            wsty_t = []
            biasQ = cpool.tile([128, 1], F32)
            wq = {}  # (s, g, d) -> [128,128] bf16

            # allocate + pre-zero all wq tiles up front (DVE is idle early)
            for _s in range(BS):
                for _g2 in range(3):
                    for _d2 in (-1, 0, 1):
                        wt = wpool.tile(
                            [128, 128], BF16, tag=f"wq{_s}_{_g2}_{_d2}"
                        )
                        if len(_BLOCKS[(_g2, _d2)]) < 4:
                            nc.vector.memset(wt[:], 0.0)
                        wq[(_s, _g2, _d2)] = wt

            def build_wq(s, styleC, part=None):
                """18 styled block writes for sample s's 9 lhsT tiles,
                fused style multiply, split across DVE and ACT.  part
                (0..2) emits only that third of the writes."""
                ops = []
                for g in range(3):
                    for d in (-1, 0, 1):
                        wt = wq[(s, g, d)]
                        for qi, qo, kx in _BLOCKS[(g, d)]:
                            dst = wt[
                                64 * qi : 64 * qi + 64, 64 * qo : 64 * qo + 64
                            ]
                            ops.append((dst, ksty0[:, 3 * g + kx, :]))
                if part is not None:
                    ops = ops[6 * part : 6 * part + 6]
                sty = styleC[:, s : s + 1]
                for flip, (dst, src) in enumerate(ops):
                    if flip % 2 == 0:
                        nc.vector.tensor_scalar_mul(dst, src, sty)
                    else:
                        nc.scalar.activation(dst, src, AF.Copy, scale=sty)

            # main-loop pools open BEFORE the prologue pool so the
            # prologue's released zone is never reused (no zone-overlap
            # dep stalling the first staging DMA)
            from contextlib import ExitStack
            _mstack = ExitStack()
            stpool = _mstack.enter_context(tc.tile_pool(name="stage", bufs=4))
            epool = _mstack.enter_context(tc.tile_pool(name="epool", bufs=4))
            opool = _mstack.enter_context(tc.tile_pool(name="onat", bufs=4))
            tpsum = _mstack.enter_context(
                tc.tile_pool(name="tpsum", bufs=2, space="PSUM"))
            qpsum = _mstack.enter_context(
                tc.tile_pool(name="qpsum", bufs=3, space="PSUM"))
            opsum = _mstack.enter_context(
                tc.tile_pool(name="opsum", bufs=2, space="PSUM"))

            # Pool queue: chunk0 first, then ksty0, then ring zeroing
            stg0_pre = stpool.tile([128, 4, 128], BF16)
            nc.gpsimd.dma_start(
                stg0_pre[:],
                x_h[0, 0:4, :, :].rearrange("r (p q) c -> p r (q c)", p=128),
            )
            nc.gpsimd.dma_start(
                ksty0[:], ck_h[:].rearrange("ky kx c f -> c (ky kx) f")
            )
            # ring zeroing (Pool): pad col, halo cols, slot 0 + pad slot
            ring_slots = ring[:, 1:RW].rearrange("p (s c) -> p s c", c=130)
            nc.gpsimd.memset(ring[:, 0:1], 0.0)
            nc.gpsimd.memset(ring_slots[:, :, 0:1], 0.0)
            nc.gpsimd.memset(ring_slots[:, :, 129:130], 0.0)
            nc.gpsimd.memset(ring[:, _scol(0) : _scol(0) + 130], 0.0)
            nc.gpsimd.memset(ring[:, _scol(NR) : _scol(NR) + 130], 0.0)
            nc.gpsimd.memset(zeroB[:], 0.0)

            # ---- prologue: style, demod, sample-0 weights ----
            # (pro/prop stay open so sample-1's demod can run mid-loop)
            pro = _mstack.enter_context(tc.tile_pool(name="pro", bufs=1))
            prop = _mstack.enter_context(
                tc.tile_pool(name="prop", bufs=1, space="PSUM"))
            if True:
                psLat = prop.tile([128, 4, BS], F32, tag="p")
                for jj in range(4):
                    nc.tensor.transpose(
                        psLat[:, jj, :],
                        latR[:, 128 * jj : 128 * jj + 128],
                        identD[0:BS, 0:BS],
                    )
                latT = pro.tile([128, 4, BS], F32)
                nc.vector.tensor_copy(latT[:], psLat[:])
                ps_style = prop.tile([CIN, BS], F32, tag="p")
                for jj in range(4):
                    nc.tensor.matmul(
                        ps_style[:],
                        dwT[:, jj, :],
                        latT[:, jj, :],
                        start=(jj == 0),
                        stop=(jj == 3),
                    )
                db_t = pro.tile([CIN, 1], F32)
                nc.sync.dma_start(
                    db_t[:], db_h[:].rearrange("(c u) -> c u", u=1)
                )
                db_s = pro.tile([CIN, 1], F32)
                nc.vector.tensor_scalar_mul(db_s[:], db_t[:], const_k)
                styleC = pro.tile([64, BS], F32)
                nc.vector.tensor_scalar(
                    styleC[:],
                    ps_style[:],
                    inv_sqrt_lat * const_k,
                    db_s[:],
                    op0=ALU.mult,
                    op1=ALU.add,
                )

                bi_t = pro.tile([64, 1], F32)
                nc.sync.dma_start(
                    bi_t[:], bi_h[:].rearrange("(c u) -> c u", u=1)
                )
                nc.vector.tensor_copy(biasQ[0:64, :], bi_t[:])
                nc.sync.dma_start(biasQ[64:128, :], biasQ[0:64, :])
                scrB = pro.tile([64, 1], F32)
                nc.vector.tensor_copy(scrB[:], biasQ[64:128, :])

            def build_demod(s):
                wsty = cpool.tile([64, 9, F], F32, tag=f"wsty{s}")
                nc.vector.tensor_scalar_mul(
                    wsty[:], ksty0[:], styleC[:, s : s + 1]
                )
                wsty_t.append(wsty)
                # demod = rsqrt(sum wsty^2 + eps)
                sq = pro.tile([64, 9, F], F32, tag=f"sq{s}")
                nc.vector.tensor_mul(sq[:], wsty[:], wsty[:])
                ps_d = prop.tile([CIN, 1], F32, tag="p")
                for t9 in range(9):
                    nc.tensor.matmul(
                        ps_d[:],
                        sq[:, t9, :],
                        ones64[:],
                        start=(t9 == 0),
                        stop=False,
                    )
                nc.tensor.matmul(
                    ps_d[:], epsT[:], ones64[:], start=False, stop=True
                )
                rt = pro.tile([64, 1], F32, tag=f"rt{s}")
                nc.scalar.activation(rt[:], ps_d[:], AF.Sqrt, bias=zero64[:])
                dm = cpool.tile([128, 1], F32, tag=f"demod{s}")
                nc.vector.reciprocal(dm[0:64, :], rt[:])
                nc.sync.dma_start(dm[64:128, :], dm[0:64, :])
                scrD = pro.tile([64, 1], F32, tag=f"scrD{s}")
                nc.vector.tensor_copy(scrD[:], dm[64:128, :])
                demodQ.append(dm)

            build_wq(0, styleC)
            build_demod(0)

            # ---- main loop ----
            if True:
                for s in range(BS):
                    if s == 0:
                        stg0 = stg0_pre
                    else:
                        # prefetch chunk 0 (rows 0..3); Pool cast-DMA
                        stg0 = stpool.tile([128, 4, 128], BF16)
                        nc.gpsimd.dma_start(
                            stg0[:],
                            x_h[s, 0:4, :, :].rearrange(
                                "r (p q) c -> p r (q c)", p=128
                            ),
                        )
                    stgs = {0: stg0}
                    qtiles = {}

                    for t in range(131):
                        # -- input: rows 2t, 2t+1 --
                        if t <= 127:
                            if t % 2 == 0 and t + 2 <= 127:
                                k = t // 2 + 1
                                stgk = stpool.tile([128, 4, 128], BF16)
                                nc.gpsimd.dma_start(
                                    stgk[:],
                                    x_h[s, 4 * k : 4 * k + 4, :, :].rearrange(
                                        "r (p q) c -> p r (q c)", p=128
                                    ),
                                )
                                stgs[k] = stgk
                                stgs.pop(k - 2, None)
                            chunk = stgs[t // 2]
                            r0 = 2 * (t % 2)
                            pt = tpsum.tile([128, 2, 128], BF16)
                            for r in range(2):
                                nc.tensor.transpose(
                                    pt[:, r, :], chunk[:, r0 + r, :], identB[:]
                                )
                            for r in range(2):
                                m = 2 * t + r
                                sl = _slot(m, s)
                                nc.scalar.activation(
                                    ring[:, _scol(sl) + 1 : _scol(sl) + 129],
                                    pt[:, r, :],
                                    AF.Copy,
                                )
                                if sl == 0:
                                    nc.scalar.activation(
                                        ring[
                                            :,
                                            _scol(NR) + 1 : _scol(NR) + 129,
                                        ],
                                        pt[:, r, :],
                                        AF.Copy,
                                    )
                        elif t == 128:
                            # zero row 256
                            sl = _slot(256, s)
                            nc.scalar.activation(
                                ring[:, _scol(sl) + 1 : _scol(sl) + 129],
                                zeroB[:],
                                AF.Copy,
                            )
                            if sl == 0:
                                nc.scalar.activation(
                                    ring[:, _scol(NR) + 1 : _scol(NR) + 129],
                                    zeroB[:],
                                    AF.Copy,
                                )

                        if s == 0 and 20 <= t <= 22:
                            # sample-1 weights spread over s0's shadow
                            build_wq(1, styleC, part=t - 20)
                        elif s == 0 and t == 23:
                            build_demod(1)

                        # -- matmuls for Q_{t-2} --
                        jmm = t - 2
                        if 0 <= jmm <= 127:
                            Q = qpsum.tile([128, 260], F32, tag="q")
                            first = True
                            for g in range(3):
                                sb = _slot(2 * jmm + g - 1, s)
                                for d in (-1, 0, 1):
                                    base = _scol(sb) + d
                                    nc.tensor.matmul(
                                        Q[:, 0:259],
                                        wq[(s, g, d)][:],
                                        ring[:, base : base + 259],
                                        start=first,
                                        stop=(g == 2 and d == 1),
                                        skip_group_check=True,
                                    )
                                    first = False
                            qtiles[jmm] = Q

                        # -- epilogue for Q_{t-3} --
                        je = t - 3
                        if 0 <= je <= 127:
                            Q = qtiles.pop(je)
                            E = epool.tile([128, 260], BF16)
                            nc.vector.tensor_scalar(
                                E[:, 0:259],
                                Q[:, 0:259],
                                demodQ[s][:],
                                biasQ[:],
                                op0=ALU.mult,
                                op1=ALU.add,
                            )
                            po = opsum.tile([128, 2, 128], BF16)
                            for r in range(2):
                                nc.tensor.transpose(
                                    po[:, r, :],
                                    E[:, 130 * r + 1 : 130 * r + 129],
                                    identB[:],
                                )
                            onat = opool.tile([128, 2, 128], F32)
                            nc.vector.tensor_copy(onat[:], po[:])
                            nc.sync.dma_start(
                                out_h[s, 2 * je : 2 * je + 2, :, :].rearrange(
                                    "r (p q) f -> p r (q f)", p=128
                                ),
                                onat[:],
                            )

            _mstack.close()

    nc.compile()
    return nc


def _get_nc():
    if "nc" not in _CACHE:
        _CACHE["nc"] = _build_nc()
    return _CACHE["nc"]


def kernel(feature_map, latent, dense_w, dense_b, conv_kernel, bias):
    nc = _get_nc()
    feature_map = np.ascontiguousarray(feature_map, dtype=np.float32)
    latent = np.ascontiguousarray(latent, dtype=np.float32)
    ident = np.eye(128, dtype=np.float32)
    in_maps = []
    for i in range(NCORES):
        in_maps.append(
            {
                "x": np.ascontiguousarray(feature_map[BS * i : BS * (i + 1)]),
                "lat": np.ascontiguousarray(latent[BS * i : BS * (i + 1)]),
                "dw": np.ascontiguousarray(dense_w, dtype=np.float32),
                "db": np.ascontiguousarray(dense_b, dtype=np.float32),
                "ck": np.ascontiguousarray(conv_kernel, dtype=np.float32),
                "bi": np.ascontiguousarray(bias, dtype=np.float32),
                "ident": ident,
            }
        )
    res = run_bass_kernel_spmd(nc, in_maps, core_ids=list(range(NCORES)))
    outs = [res.results[i]["out"] for i in range(NCORES)]
    full = np.concatenate(outs, axis=0)
    if getattr(res, "exec_time_ns", None):
        kernel.last_exec_time_ns = res.exec_time_ns
    return full


kernel.last_exec_time_ns = None



# revision 3
# speedup vs baseline: 1.1269x; 1.1269x over previous
"""Demodulated 3x3 convolution Trainium2 kernel — data-stationary odd-pair
scheme.

Data-parallel over batch: 16 samples -> 8 cores x 2 samples.

Input path: HBM NHWC rows are fetched in 8-row chunks with partition =
w-PAIR (512B contiguous descriptors), Pool/SWDGE DMA casting fp32->bf16
in flight.  PE-transpose turns each row into a channel-major slab
[(q,c) 128 parts, 128 wpair cols]; DVE/ACT copies place it into a
non-circular 258-slot ring with ODD-PAIR lanes: slot sl holds image row
sl-1 as ring[(d,c), 129*sl + u] = x[row, 2u+d-1, c] (parts 0:64 = d=1
lane at col offset 0, parts 64:128 = d=0 lane at offset +1; w=-1/256
halo cols and slots 0/257 are statically zero).

Conv (data-stationary): for output row m, wpair j, out[j, (qo,f)] is
accumulated by 6 matmuls (3 ky-groups g x 2 shifts v) with lhsT =
ring[:, 129*(m+g)+v : +128] (the DATA is stationary; Ldweights are free
in the cost model) and rhs = per-sample weight tile wq[s,g,v]
[(d,c), (qo,f)] holding styled+demodulated tap W[g, kx], kx = 2v+d-qo
when 0<=kx<=2 (3 of 4 blocks per tile; 18 blocks over 6 tiles = the
provable minimum for this tap/domino structure).  N=128 per matmul,
PE/iter = 12 matmuls + 2 transposes = 1792 cyc.

Demodulation is folded into the weights: wq = W * const_k * style[c] *
demod[f] built per-sample on DVE (demod[f] broadcast across partitions
via a PE transpose of a column-replicated rsqrt).  The epilogue is a
single ACT copy Q'[j, 2, (qo,f)] PSUM fp32 -> SBUF, and the SP output
DMA writes 512B descriptors.  Conv bias is zero for this problem; a
nonzero bias selects a variant adding a K=1 matmul per row.

Per-iteration budget (cost model): PE 747ns, ACT ~690, DVE ~583,
DMA engines ~728, HWDGE 625, Pool ~335, SP ~590.
"""

import math
import os
import sys

os.environ["BASS_NEVER_TRACE"] = "1"

for _p in ("/opt/trn_rl_repo",):
    if _p not in sys.path:
        sys.path.insert(0, _p)

import numpy as np

import concourse.bass as bass
import concourse.bacc as bacc
import concourse.mybir as mybir
import concourse.tile as tile
from concourse.bass_utils import run_bass_kernel_spmd

B, H, W, CIN = 16, 256, 256, 64
LATENT, F, KK = 512, 64, 3
NCORES = 8
BS = B // NCORES  # samples per core
SLOT = 129  # ring cols per slot (u = 0..128)
NSLOT = H + 2  # slots 0..257; slot sl holds image row sl-1
RW = SLOT * NSLOT

F32 = mybir.dt.float32
BF16 = mybir.dt.bfloat16
AF = mybir.ActivationFunctionType
ALU = mybir.AluOpType

_CACHE = {}


def _build_nc(bias_nonzero):
    nc = bacc.Bacc("TRN2", target_bir_lowering=False, debug=False)

    x_h = nc.dram_tensor("x", [BS, H, W, CIN], F32, kind="ExternalInput")
    lat_h = nc.dram_tensor("lat", [BS, LATENT], F32, kind="ExternalInput")
    dw_h = nc.dram_tensor("dw", [LATENT, CIN], F32, kind="ExternalInput")
    db_h = nc.dram_tensor("db", [CIN], F32, kind="ExternalInput")
    ck_h = nc.dram_tensor("ck", [KK, KK, CIN, F], F32, kind="ExternalInput")
    bi_h = nc.dram_tensor("bi", [F], F32, kind="ExternalInput")
    id_h = nc.dram_tensor("ident", [128, 128], F32, kind="ExternalInput")
    out_h = nc.dram_tensor("out", [BS, H, W, F], F32, kind="ExternalOutput")

    const_k = math.sqrt(2.0) / math.sqrt(KK * KK * CIN)
    inv_sqrt_lat = 1.0 / math.sqrt(LATENT)

    with tile.TileContext(nc) as tc:
        with (
            tc.tile_pool(name="const", bufs=1) as cpool,
            tc.tile_pool(name="wpool", bufs=1) as wpool,
        ):
            # --- loads: chunk0+ksty0 on Pool; ident/lat/dw on SP ---
            identD = cpool.tile([128, 128], F32)
            nc.sync.dma_start(identD[:], id_h[:])
            latR = cpool.tile([BS, LATENT], F32)
            nc.sync.dma_start(latR[:], lat_h[:])
            dwD = cpool.tile([128, 4, CIN], F32)
            nc.sync.dma_start(
                dwD[:], dw_h[:].rearrange("(j p) f -> p j f", p=128)
            )
            ksty0 = cpool.tile([64, 9, F], F32)

            # DVE: matmul operand copies first so style matmuls start early
            dwT = cpool.tile([128, 4, CIN], F32)
            nc.vector.tensor_copy(dwT[:], dwD[:])
            identB = cpool.tile([128, 128], BF16)
            nc.vector.tensor_copy(identB[:], identD[:])
            ones64 = cpool.tile([64, 1], F32)
            nc.vector.memset(ones64[:], 1.0)
            zero64 = cpool.tile([64, 1], F32)
            nc.vector.memset(zero64[:], 0.0)
            onesT64 = cpool.tile([64, 64], F32)
            nc.vector.memset(onesT64[:], 1.0)
            epsT = cpool.tile([64, F], F32)
            nc.vector.memset(epsT[:], 1e-8 / 64.0)

            ring = cpool.tile([128, RW], BF16)

            wq = {}  # (s, g, v) -> [128,128] bf16 lhs... rhs weight tile
            for _s in range(BS):
                for _g in range(3):
                    for _v in range(2):
                        wt = wpool.tile(
                            [128, 128], BF16, tag=f"wq{_s}_{_g}_{_v}"
                        )
                        wq[(_s, _g, _v)] = wt

            # main-loop pools open BEFORE the prologue pool so the
            # prologue's released zone is never reused
            from contextlib import ExitStack
            _mstack = ExitStack()
            stpool = _mstack.enter_context(tc.tile_pool(name="stage", bufs=3))
            opool = _mstack.enter_context(tc.tile_pool(name="onat", bufs=4))
            tpsum = _mstack.enter_context(
                tc.tile_pool(name="tpsum", bufs=2, space="PSUM"))
            qpsum = _mstack.enter_context(
                tc.tile_pool(name="qpsum", bufs=3, space="PSUM"))

            # Pool queue: chunk0 first, then ksty0, then ring zeroing
            stg0_pre = stpool.tile([128, 8, 128], BF16)
            nc.gpsimd.dma_start(
                stg0_pre[:],
                x_h[0, 0:8, :, :].rearrange("r (p q) c -> p r (q c)", p=128),
            )
            nc.gpsimd.dma_start(
                ksty0[:], ck_h[:].rearrange("ky kx c f -> c (ky kx) f")
            )
            # static ring zeroing: halo cols of every slot + slots 0/257
            ring_v = ring[:].rearrange("p (s c) -> p s c", c=SLOT)
            nc.gpsimd.memset(ring_v[0:64, :, 128:129], 0.0)
            nc.gpsimd.memset(ring_v[64:128, :, 0:1], 0.0)
            nc.gpsimd.memset(ring[:, 0:SLOT], 0.0)
            nc.gpsimd.memset(ring[:, SLOT * (NSLOT - 1) : SLOT * NSLOT], 0.0)

            # ---- prologue: style for both samples ----
            pro = _mstack.enter_context(tc.tile_pool(name="pro", bufs=1))
            prop = _mstack.enter_context(
                tc.tile_pool(name="prop", bufs=1, space="PSUM"))

            psLat = prop.tile([128, 4, BS], F32, tag="p")
            for jj in range(4):
                nc.tensor.transpose(
                    psLat[:, jj, :],
                    latR[:, 128 * jj : 128 * jj + 128],
                    identD[0:BS, 0:BS],
                )
            latT = pro.tile([128, 4, BS], F32)
            nc.vector.tensor_copy(latT[:], psLat[:])
            ps_style = prop.tile([CIN, BS], F32, tag="p")
            for jj in range(4):
                nc.tensor.matmul(
                    ps_style[:],
                    dwT[:, jj, :],
                    latT[:, jj, :],
                    start=(jj == 0),
                    stop=(jj == 3),
                )
            db_t = pro.tile([CIN, 1], F32)
            nc.sync.dma_start(db_t[:], db_h[:].rearrange("(c u) -> c u", u=1))
            db_s = pro.tile([CIN, 1], F32)
            nc.vector.tensor_scalar_mul(db_s[:], db_t[:], const_k)
            styleC = pro.tile([64, BS], F32)
            nc.vector.tensor_scalar(
                styleC[:],
                ps_style[:],
                inv_sqrt_lat * const_k,
                db_s[:],
                op0=ALU.mult,
                op1=ALU.add,
            )

            if bias_nonzero:
                onesRow = pro.tile([1, 128], F32)
                nc.vector.memset(onesRow[:], 1.0)
                biasPair = pro.tile([1, 128], F32)
                nc.sync.dma_start(
                    biasPair[0:1, 0:64],
                    bi_h[:].rearrange("(u c) -> u c", u=1),
                )
                nc.sync.dma_start(
                    biasPair[0:1, 64:128],
                    bi_h[:].rearrange("(u c) -> u c", u=1),
                )

            def weight_ops(s):
                """List of thunks building sample s's 6 styled+demodulated
                weight tiles.  Run inline for s=0; dribbled one per
                iteration into s0's loop for s=1."""
                ops = []
                # zero blocks first (no deps)
                for g in range(3):
                    ops.append(lambda g=g: nc.vector.memset(
                        wq[(s, g, 0)][64:128, 64:128], 0.0))
                    ops.append(lambda g=g: nc.vector.memset(
                        wq[(s, g, 1)][0:64, 0:64], 0.0))

                wsty = cpool.tile([64, 9, F], F32, tag=f"wsty{s}")
                sq = pro.tile([64, 9, F], F32, tag=f"sq{s}")
                ps_d = prop.tile([CIN, 1], F32, tag="p")
                rt = pro.tile([64, 1], F32, tag=f"rt{s}")
                dm = pro.tile([64, 1], F32, tag=f"dm{s}")
                repl = pro.tile([64, 64], F32, tag=f"repl{s}")
                replT = prop.tile([64, 64], F32, tag=f"replT{s}")
                demodB = pro.tile([64, 64], F32, tag=f"demodB{s}")
                w2 = cpool.tile([64, 9, F], F32, tag=f"w2_{s}")

                ops.append(lambda: nc.vector.tensor_scalar_mul(
                    wsty[:], ksty0[:], styleC[:, s : s + 1]))
                ops.append(lambda: nc.vector.tensor_mul(
                    sq[:], wsty[:], wsty[:]))

                def _psd():
                    for t9 in range(9):
                        nc.tensor.matmul(
                            ps_d[:], sq[:, t9, :], ones64[:],
                            start=(t9 == 0), stop=False,
                        )
                    nc.tensor.matmul(
                        ps_d[:], epsT[:], ones64[:], start=False, stop=True
                    )
                ops.append(_psd)

                def _rt_dm():
                    nc.scalar.activation(rt[:], ps_d[:], AF.Sqrt,
                                         bias=zero64[:])
                    nc.vector.reciprocal(dm[:], rt[:])
                ops.append(_rt_dm)
                # demod[f] broadcast across partitions: repl[c,f]=dm[c],
                # transpose -> replT[p,n]=dm[n], copy to SBUF
                ops.append(lambda: nc.scalar.activation(
                    repl[:], onesT64[:], AF.Copy, scale=dm[:]))

                def _replT():
                    nc.tensor.transpose(replT[:], repl[:],
                                        identD[0:64, 0:64])
                    nc.vector.tensor_copy(demodB[:], replT[:])
                ops.append(_replT)

                for t9 in range(9):
                    ops.append(lambda t9=t9: nc.vector.tensor_mul(
                        w2[:, t9, :], wsty[:, t9, :], demodB[:]))

                # 18 block writes: wq[(s,g,v)] blocks <- w2[:, 3g+kx, :]
                # kx = 2v + d - qo; parts 0:64 = d1, 64:128 = d0
                blocks = []
                for g in range(3):
                    blocks += [
                        (wq[(s, g, 0)][0:64, 0:64], 3 * g + 1),
                        (wq[(s, g, 0)][0:64, 64:128], 3 * g + 0),
                        (wq[(s, g, 0)][64:128, 0:64], 3 * g + 0),
                        (wq[(s, g, 1)][0:64, 64:128], 3 * g + 2),
                        (wq[(s, g, 1)][64:128, 0:64], 3 * g + 2),
                        (wq[(s, g, 1)][64:128, 64:128], 3 * g + 1),
                    ]
                for dst, idx in blocks:
                    ops.append(lambda dst=dst, idx=idx: nc.vector.tensor_copy(
                        dst, w2[:, idx, :]))
                return ops

            for op in weight_ops(0):
                op()

            # ---- main loop ----
            for s in range(BS):
                if s == 0:
                    stg0 = stg0_pre
                    shadow = weight_ops(1)
                else:
                    stg0 = stpool.tile([128, 8, 128], BF16)
                    nc.gpsimd.dma_start(
                        stg0[:],
                        x_h[s, 0:8, :, :].rearrange(
                            "r (p q) c -> p r (q c)", p=128
                        ),
                    )
                    shadow = []
                stgs = {0: stg0}
                qtiles = {}

                for t in range(131):
                    # -- input rows 2t, 2t+1 --
                    if t <= 127:
                        k, r4 = divmod(t, 4)
                        if r4 == 0 and k + 1 <= 31:
                            stgk = stpool.tile([128, 8, 128], BF16)
                            nc.gpsimd.dma_start(
                                stgk[:],
                                x_h[s, 8 * (k + 1) : 8 * (k + 2), :, :]
                                .rearrange("r (p q) c -> p r (q c)", p=128),
                            )
                            stgs[k + 1] = stgk
                            stgs.pop(k - 1, None)
                        chunk = stgs[k]
                        pt = tpsum.tile([128, 2, 128], BF16)
                        for r in range(2):
                            nc.tensor.transpose(
                                pt[:, r, :], chunk[:, 2 * r4 + r, :],
                                identB[:],
                            )
                        for r in range(2):
                            m = 2 * t + r
                            sc = SLOT * (m + 1)
                            # lane d=1 (parts 0:64) at offset 0
                            nc.vector.tensor_copy(
                                ring[0:64, sc : sc + 128], pt[0:64, r, :]
                            )
                            # lane d=0 (parts 64:128) at offset +1
                            if r == 0:
                                nc.vector.tensor_copy(
                                    ring[64:128, sc + 1 : sc + 129],
                                    pt[64:128, r, :],
                                )
                            else:
                                nc.scalar.activation(
                                    ring[64:128, sc + 1 : sc + 129],
                                    pt[64:128, r, :],
                                    AF.Copy,
                                )

                    if s == 0 and shadow and 6 <= t:
                        shadow.pop(0)()

                    # -- matmuls for row-pair jmm = t-2 --
                    jmm = t - 2
                    if 0 <= jmm <= 127:
                        Qp = qpsum.tile([128, 2, 128], F32, tag="q")
                        for r in range(2):
                            m = 2 * jmm + r
                            first = True
                            for g in range(3):
                                base = SLOT * (m + g)
                                for v in range(2):
                                    last = (
                                        g == 2 and v == 1
                                        and not bias_nonzero
                                    )
                                    nc.tensor.matmul(
                                        Qp[:, r, :],
                                        ring[:, base + v : base + v + 128],
                                        wq[(s, g, v)][:],
                                        start=first,
                                        stop=last,
                                        skip_group_check=True,
                                    )
                                    first = False
                            if bias_nonzero:
                                nc.tensor.matmul(
                                    Qp[:, r, :],
                                    onesRow[:],
                                    biasPair[:],
                                    start=False,
                                    stop=True,
                                    skip_group_check=True,
                                )
                        qtiles[jmm] = Qp

                    # -- epilogue for je = t-3 --
                    je = t - 3
                    if 0 <= je <= 127:
                        Qe = qtiles.pop(je)
                        onat = opool.tile([128, 2, 128], F32)
                        nc.scalar.activation(onat[:], Qe[:], AF.Copy)
                        nc.sync.dma_start(
                            out_h[s, 2 * je : 2 * je + 2, :, :].rearrange(
                                "r (p q) f -> p r (q f)", p=128
                            ),
                            onat[:],
                        )

            _mstack.close()

    nc.compile()
    return nc


def _get_nc(bias_nonzero=False):
    key = bool(bias_nonzero)
    if key not in _CACHE:
        _CACHE[key] = _build_nc(key)
    return _CACHE[key]


def kernel(feature_map, latent, dense_w, dense_b, conv_kernel, bias):
    bias = np.ascontiguousarray(bias, dtype=np.float32)
    nc = _get_nc(bool(np.any(bias)))
    feature_map = np.ascontiguousarray(feature_map, dtype=np.float32)
    latent = np.ascontiguousarray(latent, dtype=np.float32)
    ident = np.eye(128, dtype=np.float32)
    in_maps = []
    for i in range(NCORES):
        in_maps.append(
            {
                "x": np.ascontiguousarray(feature_map[BS * i : BS * (i + 1)]),
                "lat": np.ascontiguousarray(latent[BS * i : BS * (i + 1)]),
                "dw": np.ascontiguousarray(dense_w, dtype=np.float32),
                "db": np.ascontiguousarray(dense_b, dtype=np.float32),
                "ck": np.ascontiguousarray(conv_kernel, dtype=np.float32),
                "bi": bias,
                "ident": ident,
            }
        )
    res = run_bass_kernel_spmd(nc, in_maps, core_ids=list(range(NCORES)))
    outs = [res.results[i]["out"] for i in range(NCORES)]
    full = np.concatenate(outs, axis=0)
    if getattr(res, "exec_time_ns", None):
        kernel.last_exec_time_ns = res.exec_time_ns
    return full


kernel.last_exec_time_ns = None


# revision 5
# speedup vs baseline: 1.2576x; 1.1160x over previous
"""Demodulated 3x3 convolution Trainium2 kernel — data-stationary odd-pair
scheme.

Data-parallel over batch: 16 samples -> 8 cores x 2 samples.

Input path: HBM NHWC rows are fetched in 8-row chunks with partition =
w-PAIR (512B contiguous descriptors), Pool/SWDGE DMA casting fp32->bf16
in flight.  PE-transpose turns each row into a channel-major slab
[(q,c) 128 parts, 128 wpair cols]; DVE/ACT copies place it into a
non-circular 258-slot ring with ODD-PAIR lanes: slot sl holds image row
sl-1 as ring[(d,c), 129*sl + u] = x[row, 2u+d-1, c] (parts 0:64 = d=1
lane at col offset 0, parts 64:128 = d=0 lane at offset +1; w=-1/256
halo cols and slots 0/257 are statically zero).

Conv (data-stationary): for output row m, wpair j, out[j, (qo,f)] is
accumulated by 6 matmuls (3 ky-groups g x 2 shifts v) with lhsT =
ring[:, 129*(m+g)+v : +128] (the DATA is stationary; Ldweights are free
in the cost model) and rhs = per-sample weight tile wq[s,g,v]
[(d,c), (qo,f)] holding styled+demodulated tap W[g, kx], kx = 2v+d-qo
when 0<=kx<=2 (3 of 4 blocks per tile; 18 blocks over 6 tiles = the
provable minimum for this tap/domino structure).  N=128 per matmul,
PE/iter = 12 matmuls + 2 transposes = 1792 cyc.

Demodulation is folded into the weights: wq = W * const_k * style[c] *
demod[f] built per-sample on DVE (demod[f] broadcast across partitions
via a PE transpose of a column-replicated rsqrt).  The epilogue is a
single ACT copy Q'[j, 2, (qo,f)] PSUM fp32 -> SBUF, and the SP output
DMA writes 512B descriptors.  Conv bias is zero for this problem; a
nonzero bias selects a variant adding a K=1 matmul per row.

Per-iteration budget (cost model): PE 747ns, ACT ~690, DVE ~583,
DMA engines ~728, HWDGE 625, Pool ~335, SP ~590.
"""

import math
import os
import sys

os.environ["BASS_NEVER_TRACE"] = "1"

for _p in ("/opt/trn_rl_repo",):
    if _p not in sys.path:
        sys.path.insert(0, _p)

import numpy as np

import concourse.bass as bass
import concourse.bacc as bacc
import concourse.mybir as mybir
import concourse.tile as tile
from concourse.bass_utils import run_bass_kernel_spmd

B, H, W, CIN = 16, 256, 256, 64
LATENT, F, KK = 512, 64, 3
NCORES = 8
BS = B // NCORES  # samples per core
SLOT = 129  # ring cols per slot (u = 0..128)
NSLOT = H + 2  # slots 0..257; slot sl holds image row sl-1
RW = SLOT * NSLOT

F32 = mybir.dt.float32
BF16 = mybir.dt.bfloat16
AF = mybir.ActivationFunctionType
ALU = mybir.AluOpType

_CACHE = {}


def _build_nc(bias_nonzero):
    nc = bacc.Bacc("TRN2", target_bir_lowering=False, debug=False)

    x_h = nc.dram_tensor("x", [BS, H, W, CIN], F32, kind="ExternalInput")
    lat_h = nc.dram_tensor("lat", [BS, LATENT], F32, kind="ExternalInput")
    dw_h = nc.dram_tensor("dw", [LATENT, CIN], F32, kind="ExternalInput")
    db_h = nc.dram_tensor("db", [CIN], F32, kind="ExternalInput")
    ck_h = nc.dram_tensor("ck", [KK, KK, CIN, F], F32, kind="ExternalInput")
    bi_h = nc.dram_tensor("bi", [F], F32, kind="ExternalInput")
    id_h = nc.dram_tensor("ident", [128, 128], F32, kind="ExternalInput")
    out_h = nc.dram_tensor("out", [BS, H, W, F], F32, kind="ExternalOutput")

    const_k = math.sqrt(2.0) / math.sqrt(KK * KK * CIN)
    inv_sqrt_lat = 1.0 / math.sqrt(LATENT)

    with tile.TileContext(nc) as tc:
        with (
            tc.tile_pool(name="const", bufs=1) as cpool,
            tc.tile_pool(name="wpool", bufs=1) as wpool,
        ):
            # --- loads: chunk0+ksty0 on Pool; ident/lat/dw on SP ---
            identD = cpool.tile([128, 128], F32)
            nc.sync.dma_start(identD[:], id_h[:])
            latR = cpool.tile([BS, LATENT], F32)
            nc.sync.dma_start(latR[:], lat_h[:])
            dwD = cpool.tile([128, 4, CIN], F32)
            nc.sync.dma_start(
                dwD[:], dw_h[:].rearrange("(j p) f -> p j f", p=128)
            )
            ksty0 = cpool.tile([64, 9, F], F32)

            # DVE: matmul operand copies first so style matmuls start early
            dwT = cpool.tile([128, 4, CIN], F32)
            nc.vector.tensor_copy(dwT[:], dwD[:])
            identB = cpool.tile([128, 128], BF16)
            nc.vector.tensor_copy(identB[:], identD[:])
            ones64 = cpool.tile([64, 1], F32)
            nc.vector.memset(ones64[:], 1.0)
            zero64 = cpool.tile([64, 1], F32)
            nc.vector.memset(zero64[:], 0.0)
            onesT64 = cpool.tile([64, 64], F32)
            nc.vector.memset(onesT64[:], 1.0)
            epsT = cpool.tile([64, F], F32)
            nc.vector.memset(epsT[:], 1e-8 / 64.0)

            ring = cpool.tile([128, RW], BF16)

            wq = {}  # (s, g, v) -> [128,128] bf16 lhs... rhs weight tile
            for _s in range(BS):
                for _g in range(3):
                    for _v in range(2):
                        wt = wpool.tile(
                            [128, 128], BF16, tag=f"wq{_s}_{_g}_{_v}"
                        )
                        wq[(_s, _g, _v)] = wt

            # main-loop pools open BEFORE the prologue pool so the
            # prologue's released zone is never reused
            from contextlib import ExitStack
            _mstack = ExitStack()
            stpool = _mstack.enter_context(tc.tile_pool(name="stage", bufs=3))
            opool = _mstack.enter_context(tc.tile_pool(name="onat", bufs=4))
            tpsum = _mstack.enter_context(
                tc.tile_pool(name="tpsum", bufs=2, space="PSUM"))
            qpsum = _mstack.enter_context(
                tc.tile_pool(name="qpsum", bufs=3, space="PSUM"))

            # Pool queue: chunk0 first, then ksty0, then ring zeroing
            stg0_pre = stpool.tile([128, 8, 128], BF16)
            nc.gpsimd.dma_start(
                stg0_pre[:],
                x_h[0, 0:8, :, :].rearrange("r (p q) c -> p r (q c)", p=128),
            )
            nc.gpsimd.dma_start(
                ksty0[:], ck_h[:].rearrange("ky kx c f -> c (ky kx) f")
            )
            # static ring zeroing: halo cols of every slot + slots 0/257
            ring_v = ring[:].rearrange("p (s c) -> p s c", c=SLOT)
            nc.gpsimd.memset(ring_v[0:64, :, 128:129], 0.0)
            nc.gpsimd.memset(ring_v[64:128, :, 0:1], 0.0)
            nc.gpsimd.memset(ring[:, 0:SLOT], 0.0)
            nc.gpsimd.memset(ring[:, SLOT * (NSLOT - 1) : SLOT * NSLOT], 0.0)

            # ---- prologue: style for both samples ----
            pro = _mstack.enter_context(tc.tile_pool(name="pro", bufs=1))
            prop = _mstack.enter_context(
                tc.tile_pool(name="prop", bufs=1, space="PSUM"))

            psLat = prop.tile([128, 4, BS], F32, tag="p")
            for jj in range(4):
                nc.tensor.transpose(
                    psLat[:, jj, :],
                    latR[:, 128 * jj : 128 * jj + 128],
                    identD[0:BS, 0:BS],
                )
            latT = pro.tile([128, 4, BS], F32)
            nc.vector.tensor_copy(latT[:], psLat[:])
            ps_style = prop.tile([CIN, BS], F32, tag="p")
            for jj in range(4):
                nc.tensor.matmul(
                    ps_style[:],
                    dwT[:, jj, :],
                    latT[:, jj, :],
                    start=(jj == 0),
                    stop=(jj == 3),
                )
            db_t = pro.tile([CIN, 1], F32)
            nc.sync.dma_start(db_t[:], db_h[:].rearrange("(c u) -> c u", u=1))
            db_s = pro.tile([CIN, 1], F32)
            nc.vector.tensor_scalar_mul(db_s[:], db_t[:], const_k)
            styleC = pro.tile([64, BS], F32)
            nc.vector.tensor_scalar(
                styleC[:],
                ps_style[:],
                inv_sqrt_lat * const_k,
                db_s[:],
                op0=ALU.mult,
                op1=ALU.add,
            )

            if bias_nonzero:
                onesRow = pro.tile([1, 128], F32)
                nc.vector.memset(onesRow[:], 1.0)
                biasPair = pro.tile([1, 128], F32)
                nc.sync.dma_start(
                    biasPair[0:1, 0:64],
                    bi_h[:].rearrange("(u c) -> u c", u=1),
                )
                nc.sync.dma_start(
                    biasPair[0:1, 64:128],
                    bi_h[:].rearrange("(u c) -> u c", u=1),
                )

            def weight_ops(s):
                """List of thunks building sample s's 6 styled+demodulated
                weight tiles.  Run inline for s=0; dribbled one per
                iteration into s0's loop for s=1."""
                ops = []
                # zero blocks first (no deps)
                for g in range(3):
                    ops.append(lambda g=g: nc.vector.memset(
                        wq[(s, g, 0)][64:128, 64:128], 0.0))
                    ops.append(lambda g=g: nc.vector.memset(
                        wq[(s, g, 1)][0:64, 0:64], 0.0))

                wsty = cpool.tile([64, 9, F], F32, tag=f"wsty{s}")
                sq = pro.tile([64, 9, F], F32, tag=f"sq{s}")
                ps_d = prop.tile([CIN, 1], F32, tag="p")
                rt = pro.tile([64, 1], F32, tag=f"rt{s}")
                dm = pro.tile([64, 1], F32, tag=f"dm{s}")
                repl = pro.tile([64, 64], F32, tag=f"repl{s}")
                replT = prop.tile([64, 64], F32, tag="p")
                demodB = pro.tile([64, 64], F32, tag=f"demodB{s}")
                w2 = cpool.tile([64, 9, F], F32, tag=f"w2_{s}")

                ops.append(lambda: nc.vector.tensor_scalar_mul(
                    wsty[:], ksty0[:], styleC[:, s : s + 1]))
                ops.append(lambda: nc.vector.tensor_mul(
                    sq[:], wsty[:], wsty[:]))

                def _psd():
                    for t9 in range(9):
                        nc.tensor.matmul(
                            ps_d[:], sq[:, t9, :], ones64[:],
                            start=(t9 == 0), stop=False,
                        )
                    nc.tensor.matmul(
                        ps_d[:], epsT[:], ones64[:], start=False, stop=True
                    )
                ops.append(_psd)

                def _rt_dm():
                    nc.scalar.activation(rt[:], ps_d[:], AF.Sqrt,
                                         bias=zero64[:])
                    nc.vector.reciprocal(dm[:], rt[:])
                ops.append(_rt_dm)
                # demod[f] broadcast across partitions: repl[c,f]=dm[c],
                # transpose -> replT[p,n]=dm[n], copy to SBUF
                ops.append(lambda: nc.scalar.activation(
                    repl[:], onesT64[:], AF.Copy, scale=dm[:]))

                def _replT():
                    nc.tensor.transpose(replT[:], repl[:],
                                        identD[0:64, 0:64])
                    nc.vector.tensor_copy(demodB[:], replT[:])
                ops.append(_replT)

                for t9 in range(9):
                    ops.append(lambda t9=t9: nc.vector.tensor_mul(
                        w2[:, t9, :], wsty[:, t9, :], demodB[:]))

                # 18 block writes: wq[(s,g,v)] blocks <- w2[:, 3g+kx, :]
                # kx = 2v + d - qo; parts 0:64 = d1, 64:128 = d0
                blocks = []
                for g in range(3):
                    blocks += [
                        (wq[(s, g, 0)][0:64, 0:64], 3 * g + 1),
                        (wq[(s, g, 0)][0:64, 64:128], 3 * g + 0),
                        (wq[(s, g, 0)][64:128, 0:64], 3 * g + 0),
                        (wq[(s, g, 1)][0:64, 64:128], 3 * g + 2),
                        (wq[(s, g, 1)][64:128, 0:64], 3 * g + 2),
                        (wq[(s, g, 1)][64:128, 64:128], 3 * g + 1),
                    ]
                for dst, idx in blocks:
                    ops.append(lambda dst=dst, idx=idx: nc.vector.tensor_copy(
                        dst, w2[:, idx, :]))
                return ops

            for op in weight_ops(0):
                op()

            # ---- main loop ----
            for s in range(BS):
                if s == 0:
                    stg0 = stg0_pre
                    shadow = weight_ops(1)
                else:
                    stg0 = stpool.tile([128, 8, 128], BF16)
                    nc.gpsimd.dma_start(
                        stg0[:],
                        x_h[s, 0:8, :, :].rearrange(
                            "r (p q) c -> p r (q c)", p=128
                        ),
                    )
                    shadow = []
                stgs = {0: stg0}
                qtiles = {}

                for t in range(131):
                    # -- input rows 2t, 2t+1 --
                    if t <= 127:
                        k, r4 = divmod(t, 4)
                        if r4 == 0 and k + 1 <= 31:
                            stgk = stpool.tile([128, 8, 128], BF16)
                            nc.gpsimd.dma_start(
                                stgk[:],
                                x_h[s, 8 * (k + 1) : 8 * (k + 2), :, :]
                                .rearrange("r (p q) c -> p r (q c)", p=128),
                            )
                            stgs[k + 1] = stgk
                            stgs.pop(k - 1, None)
                        chunk = stgs[k]
                        ptA = tpsum.tile([128, 128], BF16, tag="ptA")
                        ptB = tpsum.tile([128, 128], BF16, tag="ptB")
                        nc.tensor.transpose(
                            ptA[:], chunk[:, 2 * r4, :], identB[:]
                        )
                        nc.tensor.transpose(
                            ptB[:], chunk[:, 2 * r4 + 1, :], identB[:]
                        )
                        sc0 = SLOT * (2 * t + 1)
                        sc1 = SLOT * (2 * t + 2)
                        # DVE copies alternate PSUM zones (A,B,A) — reads
                        # of the same PSUM tile back-to-back serialize on
                        # the pipeline tail (+160ns each)
                        nc.vector.tensor_copy(
                            ring[0:64, sc0 : sc0 + 128], ptA[0:64, :]
                        )
                        nc.vector.tensor_copy(
                            ring[0:64, sc1 : sc1 + 128], ptB[0:64, :]
                        )
                        nc.vector.tensor_copy(
                            ring[64:128, sc0 + 1 : sc0 + 129], ptA[64:128, :]
                        )
                        nc.scalar.activation(
                            ring[64:128, sc1 + 1 : sc1 + 129],
                            ptB[64:128, :],
                            AF.Copy,
                        )

                    if s == 0 and shadow and 6 <= t and t % 2 == 0:
                        shadow.pop(0)()

                    # -- matmuls for row-pair jmm = t-2 --
                    jmm = t - 2
                    if 0 <= jmm <= 127:
                        Qp = qpsum.tile([128, 2, 128], F32, tag="q")
                        for r in range(2):
                            m = 2 * jmm + r
                            first = True
                            for g in range(3):
                                base = SLOT * (m + g)
                                for v in range(2):
                                    last = (
                                        g == 2 and v == 1
                                        and not bias_nonzero
                                    )
                                    nc.tensor.matmul(
                                        Qp[:, r, :],
                                        ring[:, base + v : base + v + 128],
                                        wq[(s, g, v)][:],
                                        start=first,
                                        stop=last,
                                        skip_group_check=True,
                                    )
                                    first = False
                            if bias_nonzero:
                                nc.tensor.matmul(
                                    Qp[:, r, :],
                                    onesRow[:],
                                    biasPair[:],
                                    start=False,
                                    stop=True,
                                    skip_group_check=True,
                                )
                        qtiles[jmm] = Qp

                    # -- epilogue for je = t-3 --
                    je = t - 3
                    if 0 <= je <= 127:
                        Qe = qtiles.pop(je)
                        onat = opool.tile([128, 2, 128], F32)
                        nc.scalar.activation(onat[:], Qe[:], AF.Copy)
                        nc.sync.dma_start(
                            out_h[s, 2 * je : 2 * je + 2, :, :].rearrange(
                                "r (p q) f -> p r (q f)", p=128
                            ),
                            onat[:],
                        )

            _mstack.close()

    nc.compile()
    return nc


def _get_nc(bias_nonzero=False):
    key = bool(bias_nonzero)
    if key not in _CACHE:
        _CACHE[key] = _build_nc(key)
    return _CACHE[key]


def kernel(feature_map, latent, dense_w, dense_b, conv_kernel, bias):
    bias = np.ascontiguousarray(bias, dtype=np.float32)
    nc = _get_nc(bool(np.any(bias)))
    feature_map = np.ascontiguousarray(feature_map, dtype=np.float32)
    latent = np.ascontiguousarray(latent, dtype=np.float32)
    ident = np.eye(128, dtype=np.float32)
    in_maps = []
    for i in range(NCORES):
        in_maps.append(
            {
                "x": np.ascontiguousarray(feature_map[BS * i : BS * (i + 1)]),
                "lat": np.ascontiguousarray(latent[BS * i : BS * (i + 1)]),
                "dw": np.ascontiguousarray(dense_w, dtype=np.float32),
                "db": np.ascontiguousarray(dense_b, dtype=np.float32),
                "ck": np.ascontiguousarray(conv_kernel, dtype=np.float32),
                "bi": bias,
                "ident": ident,
            }
        )
    res = run_bass_kernel_spmd(nc, in_maps, core_ids=list(range(NCORES)))
    outs = [res.results[i]["out"] for i in range(NCORES)]
    full = np.concatenate(outs, axis=0)
    if getattr(res, "exec_time_ns", None):
        kernel.last_exec_time_ns = res.exec_time_ns
    return full


kernel.last_exec_time_ns = None


# revision 6
# speedup vs baseline: 1.5249x; 1.2126x over previous
"""Demodulated 3x3 convolution Trainium2 kernel — data-stationary odd-pair
scheme.

Data-parallel over batch: 16 samples -> 8 cores x 2 samples.

Input path: HBM NHWC rows are fetched in 8-row chunks with partition =
w-PAIR (512B contiguous descriptors), Pool/SWDGE DMA casting fp32->bf16
in flight.  PE-transpose turns each row into a channel-major slab
[(q,c) 128 parts, 128 wpair cols]; DVE/ACT copies place it into a
non-circular 258-slot ring with ODD-PAIR lanes: slot sl holds image row
sl-1 as ring[(d,c), 129*sl + u] = x[row, 2u+d-1, c] (parts 0:64 = d=1
lane at col offset 0, parts 64:128 = d=0 lane at offset +1; w=-1/256
halo cols and slots 0/257 are statically zero).

Conv (data-stationary): for output row m, wpair j, out[j, (qo,f)] is
accumulated by 6 matmuls (3 ky-groups g x 2 shifts v) with lhsT =
ring[:, 129*(m+g)+v : +128] (the DATA is stationary; Ldweights are free
in the cost model) and rhs = per-sample weight tile wq[s,g,v]
[(d,c), (qo,f)] holding styled+demodulated tap W[g, kx], kx = 2v+d-qo
when 0<=kx<=2 (3 of 4 blocks per tile; 18 blocks over 6 tiles = the
provable minimum for this tap/domino structure).  N=128 per matmul,
PE/iter = 12 matmuls + 2 transposes = 1792 cyc.

Demodulation is folded into the weights: wq = W * const_k * style[c] *
demod[f] built per-sample on DVE (demod[f] broadcast across partitions
via a PE transpose of a column-replicated rsqrt).  The epilogue is a
single ACT copy Q'[j, 2, (qo,f)] PSUM fp32 -> SBUF, and the SP output
DMA writes 512B descriptors.  Conv bias is zero for this problem; a
nonzero bias selects a variant adding a K=1 matmul per row.

Per-iteration budget (cost model): PE 747ns, ACT ~690, DVE ~583,
DMA engines ~728, HWDGE 625, Pool ~335, SP ~590.
"""

import math
import os
import sys

os.environ["BASS_NEVER_TRACE"] = "1"

for _p in ("/opt/trn_rl_repo",):
    if _p not in sys.path:
        sys.path.insert(0, _p)

import numpy as np

import concourse.bass as bass
import concourse.bacc as bacc
import concourse.mybir as mybir
import concourse.tile as tile
from concourse.bass_utils import run_bass_kernel_spmd

B, H, W, CIN = 16, 256, 256, 64
LATENT, F, KK = 512, 64, 3
NCORES = 8
BS = B // NCORES  # samples per core
SLOT = 129  # ring cols per slot (u = 0..128)
NSLOT = H + 2  # slots 0..257; slot sl holds image row sl-1
RW = SLOT * NSLOT

F32 = mybir.dt.float32
BF16 = mybir.dt.bfloat16
AF = mybir.ActivationFunctionType
ALU = mybir.AluOpType

_CACHE = {}


def _build_nc(bias_nonzero):
    nc = bacc.Bacc("TRN2", target_bir_lowering=False, debug=False)

    x_h = nc.dram_tensor("x", [BS, H, W, CIN], F32, kind="ExternalInput")
    lat_h = nc.dram_tensor("lat", [BS, LATENT], F32, kind="ExternalInput")
    dw_h = nc.dram_tensor("dw", [LATENT, CIN], F32, kind="ExternalInput")
    db_h = nc.dram_tensor("db", [CIN], F32, kind="ExternalInput")
    ck_h = nc.dram_tensor("ck", [KK, KK, CIN, F], F32, kind="ExternalInput")
    bi_h = nc.dram_tensor("bi", [F], F32, kind="ExternalInput")
    id_h = nc.dram_tensor("ident", [128, 128], F32, kind="ExternalInput")
    out_h = nc.dram_tensor("out", [BS, H, W, F], F32, kind="ExternalOutput")

    const_k = math.sqrt(2.0) / math.sqrt(KK * KK * CIN)
    inv_sqrt_lat = 1.0 / math.sqrt(LATENT)

    with tile.TileContext(nc) as tc:
        with (
            tc.tile_pool(name="const", bufs=1) as cpool,
            tc.tile_pool(name="wpool", bufs=1) as wpool,
        ):
            # --- loads: chunk0+ksty0 on Pool; ident/lat/dw on SP ---
            identD = cpool.tile([128, 128], F32)
            nc.sync.dma_start(identD[:], id_h[:])
            latR = cpool.tile([BS, LATENT], F32)
            nc.sync.dma_start(latR[:], lat_h[:])
            dwD = cpool.tile([128, 4, CIN], F32)
            nc.sync.dma_start(
                dwD[:], dw_h[:].rearrange("(j p) f -> p j f", p=128)
            )
            ksty0 = cpool.tile([64, 9, F], F32)

            # DVE: matmul operand copies first so style matmuls start early
            dwT = cpool.tile([128, 4, CIN], F32)
            nc.vector.tensor_copy(dwT[:], dwD[:])
            identB = cpool.tile([128, 128], BF16)
            nc.vector.tensor_copy(identB[:], identD[:])
            ones64 = cpool.tile([64, 1], F32)
            nc.vector.memset(ones64[:], 1.0)
            zero64 = cpool.tile([64, 1], F32)
            nc.vector.memset(zero64[:], 0.0)
            onesT64 = cpool.tile([64, 64], F32)
            nc.vector.memset(onesT64[:], 1.0)
            epsT = cpool.tile([64, F], F32)
            nc.vector.memset(epsT[:], 1e-8 / 64.0)

            ring = cpool.tile([128, RW], BF16)

            wq = {}  # (s, g, v) -> [128,128] bf16 lhs... rhs weight tile
            for _s in range(BS):
                for _g in range(3):
                    for _v in range(2):
                        wt = wpool.tile(
                            [128, 128], BF16, tag=f"wq{_s}_{_g}_{_v}"
                        )
                        wq[(_s, _g, _v)] = wt

            # main-loop pools open BEFORE the prologue pool so the
            # prologue's released zone is never reused
            from contextlib import ExitStack
            _mstack = ExitStack()
            stpool = _mstack.enter_context(tc.tile_pool(name="stage", bufs=4))
            opool = _mstack.enter_context(tc.tile_pool(name="onat", bufs=6))
            tpsum = _mstack.enter_context(
                tc.tile_pool(name="tpsum", bufs=2, space="PSUM"))
            qpsum = _mstack.enter_context(
                tc.tile_pool(name="qpsum", bufs=3, space="PSUM"))

            # Pool queue: chunk0 first, then ksty0, then ring zeroing
            stg0_pre = stpool.tile([128, 8, 128], BF16)
            nc.gpsimd.dma_start(
                stg0_pre[:],
                x_h[0, 0:8, :, :].rearrange("r (p q) c -> p r (q c)", p=128),
            )
            nc.gpsimd.dma_start(
                ksty0[:], ck_h[:].rearrange("ky kx c f -> c (ky kx) f")
            )
            # static ring zeroing: halo cols of every slot + slots 0/257
            ring_v = ring[:].rearrange("p (s c) -> p s c", c=SLOT)
            nc.gpsimd.memset(ring_v[0:64, :, 128:129], 0.0)
            nc.gpsimd.memset(ring_v[64:128, :, 0:1], 0.0)
            nc.gpsimd.memset(ring[:, 0:SLOT], 0.0)
            nc.gpsimd.memset(ring[:, SLOT * (NSLOT - 1) : SLOT * NSLOT], 0.0)

            # ---- prologue: style for both samples ----
            pro = _mstack.enter_context(tc.tile_pool(name="pro", bufs=1))
            prop = _mstack.enter_context(
                tc.tile_pool(name="prop", bufs=1, space="PSUM"))

            psLat = prop.tile([128, 4, BS], F32, tag="p")
            for jj in range(4):
                nc.tensor.transpose(
                    psLat[:, jj, :],
                    latR[:, 128 * jj : 128 * jj + 128],
                    identD[0:BS, 0:BS],
                )
            latT = pro.tile([128, 4, BS], F32)
            nc.vector.tensor_copy(latT[:], psLat[:])
            ps_style = prop.tile([CIN, BS], F32, tag="p")
            for jj in range(4):
                nc.tensor.matmul(
                    ps_style[:],
                    dwT[:, jj, :],
                    latT[:, jj, :],
                    start=(jj == 0),
                    stop=(jj == 3),
                )
            db_t = pro.tile([CIN, 1], F32)
            nc.sync.dma_start(db_t[:], db_h[:].rearrange("(c u) -> c u", u=1))
            db_s = pro.tile([CIN, 1], F32)
            nc.vector.tensor_scalar_mul(db_s[:], db_t[:], const_k)
            styleC = pro.tile([64, BS], F32)
            nc.vector.tensor_scalar(
                styleC[:],
                ps_style[:],
                inv_sqrt_lat * const_k,
                db_s[:],
                op0=ALU.mult,
                op1=ALU.add,
            )

            if bias_nonzero:
                onesRow = pro.tile([1, 128], F32)
                nc.vector.memset(onesRow[:], 1.0)
                biasPair = pro.tile([1, 128], F32)
                nc.sync.dma_start(
                    biasPair[0:1, 0:64],
                    bi_h[:].rearrange("(u c) -> u c", u=1),
                )
                nc.sync.dma_start(
                    biasPair[0:1, 64:128],
                    bi_h[:].rearrange("(u c) -> u c", u=1),
                )

            def weight_ops(s):
                """List of thunks building sample s's 6 styled+demodulated
                weight tiles.  Run inline for s=0; dribbled one per
                iteration into s0's loop for s=1."""
                ops = []
                # zero blocks first (no deps)
                for g in range(3):
                    ops.append(lambda g=g: nc.vector.memset(
                        wq[(s, g, 0)][64:128, 64:128], 0.0))
                    ops.append(lambda g=g: nc.vector.memset(
                        wq[(s, g, 1)][0:64, 0:64], 0.0))

                wsty = cpool.tile([64, 9, F], F32, tag=f"wsty{s}")
                sq = pro.tile([64, 9, F], F32, tag=f"sq{s}")
                ps_d = prop.tile([CIN, 1], F32, tag="p")
                rt = pro.tile([64, 1], F32, tag=f"rt{s}")
                dm = pro.tile([64, 1], F32, tag=f"dm{s}")
                repl = pro.tile([64, 64], F32, tag=f"repl{s}")
                replT = prop.tile([64, 64], F32, tag="p")
                demodB = pro.tile([64, 64], F32, tag=f"demodB{s}")
                w2 = cpool.tile([64, 9, F], F32, tag=f"w2_{s}")

                ops.append(lambda: nc.vector.tensor_scalar_mul(
                    wsty[:], ksty0[:], styleC[:, s : s + 1]))
                ops.append(lambda: nc.vector.tensor_mul(
                    sq[:], wsty[:], wsty[:]))

                def _psd():
                    for t9 in range(9):
                        nc.tensor.matmul(
                            ps_d[:], sq[:, t9, :], ones64[:],
                            start=(t9 == 0), stop=False,
                        )
                    nc.tensor.matmul(
                        ps_d[:], epsT[:], ones64[:], start=False, stop=True
                    )
                ops.append(_psd)

                def _rt_dm():
                    nc.scalar.activation(rt[:], ps_d[:], AF.Sqrt,
                                         bias=zero64[:])
                    nc.vector.reciprocal(dm[:], rt[:])
                ops.append(_rt_dm)
                # demod[f] broadcast across partitions: repl[c,f]=dm[c],
                # transpose -> replT[p,n]=dm[n], copy to SBUF
                ops.append(lambda: nc.scalar.activation(
                    repl[:], onesT64[:], AF.Copy, scale=dm[:]))

                def _replT():
                    nc.tensor.transpose(replT[:], repl[:],
                                        identD[0:64, 0:64])
                    nc.vector.tensor_copy(demodB[:], replT[:])
                ops.append(_replT)

                for t9 in range(9):
                    ops.append(lambda t9=t9: nc.vector.tensor_mul(
                        w2[:, t9, :], wsty[:, t9, :], demodB[:]))

                # 18 block writes: wq[(s,g,v)] blocks <- w2[:, 3g+kx, :]
                # kx = 2v + d - qo; parts 0:64 = d1, 64:128 = d0
                blocks = []
                for g in range(3):
                    blocks += [
                        (wq[(s, g, 0)][0:64, 0:64], 3 * g + 1),
                        (wq[(s, g, 0)][0:64, 64:128], 3 * g + 0),
                        (wq[(s, g, 0)][64:128, 0:64], 3 * g + 0),
                        (wq[(s, g, 1)][0:64, 64:128], 3 * g + 2),
                        (wq[(s, g, 1)][64:128, 0:64], 3 * g + 2),
                        (wq[(s, g, 1)][64:128, 64:128], 3 * g + 1),
                    ]
                for dst, idx in blocks:
                    ops.append(lambda dst=dst, idx=idx: nc.vector.tensor_copy(
                        dst, w2[:, idx, :]))
                return ops

            for op in weight_ops(0):
                op()

            # ---- main loop ----
            for s in range(BS):
                if s == 0:
                    stg0 = stg0_pre
                    shadow = weight_ops(1)
                else:
                    stg0 = stpool.tile([128, 8, 128], BF16)
                    nc.gpsimd.dma_start(
                        stg0[:],
                        x_h[s, 0:8, :, :].rearrange(
                            "r (p q) c -> p r (q c)", p=128
                        ),
                    )
                    shadow = []
                stgs = {0: stg0}
                qtiles = {}

                for t in range(131):
                    # -- input rows 2t, 2t+1 --
                    if t <= 127:
                        k, r4 = divmod(t, 4)
                        if r4 == 0 and k + 1 <= 31:
                            stgk = stpool.tile([128, 8, 128], BF16)
                            nc.gpsimd.dma_start(
                                stgk[:],
                                x_h[s, 8 * (k + 1) : 8 * (k + 2), :, :]
                                .rearrange("r (p q) c -> p r (q c)", p=128),
                            )
                            stgs[k + 1] = stgk
                            stgs.pop(k - 1, None)
                        chunk = stgs[k]
                        ptA = tpsum.tile([128, 128], BF16, tag="ptA")
                        ptB = tpsum.tile([128, 128], BF16, tag="ptB")
                        nc.tensor.transpose(
                            ptA[:], chunk[:, 2 * r4, :], identB[:]
                        )
                        nc.tensor.transpose(
                            ptB[:], chunk[:, 2 * r4 + 1, :], identB[:]
                        )
                        sc0 = SLOT * (2 * t + 1)
                        sc1 = SLOT * (2 * t + 2)
                        # DVE copies alternate PSUM zones (A,B,A) — reads
                        # of the same PSUM tile back-to-back serialize on
                        # the pipeline tail (+160ns each)
                        nc.vector.tensor_copy(
                            ring[0:64, sc0 : sc0 + 128], ptA[0:64, :]
                        )
                        nc.vector.tensor_copy(
                            ring[0:64, sc1 : sc1 + 128], ptB[0:64, :]
                        )
                        nc.vector.tensor_copy(
                            ring[64:128, sc0 + 1 : sc0 + 129], ptA[64:128, :]
                        )
                        nc.scalar.activation(
                            ring[64:128, sc1 + 1 : sc1 + 129],
                            ptB[64:128, :],
                            AF.Copy,
                        )

                    if s == 0 and shadow and 6 <= t and t % 2 == 0:
                        shadow.pop(0)()

                    # -- matmuls for row-pair jmm = t-2 --
                    jmm = t - 2
                    if 0 <= jmm <= 127:
                        Qp = qpsum.tile([128, 2, 128], F32, tag="q")
                        for r in range(2):
                            m = 2 * jmm + r
                            first = True
                            for g in range(3):
                                base = SLOT * (m + g)
                                for v in range(2):
                                    last = (
                                        g == 2 and v == 1
                                        and not bias_nonzero
                                    )
                                    nc.tensor.matmul(
                                        Qp[:, r, :],
                                        ring[:, base + v : base + v + 128],
                                        wq[(s, g, v)][:],
                                        start=first,
                                        stop=last,
                                        skip_group_check=True,
                                    )
                                    first = False
                            if bias_nonzero:
                                nc.tensor.matmul(
                                    Qp[:, r, :],
                                    onesRow[:],
                                    biasPair[:],
                                    start=False,
                                    stop=True,
                                    skip_group_check=True,
                                )
                        qtiles[jmm] = Qp

                    # -- epilogue for je = t-3 --
                    je = t - 3
                    if 0 <= je <= 127:
                        Qe = qtiles.pop(je)
                        onat = opool.tile([128, 2, 128], F32)
                        nc.scalar.activation(onat[:], Qe[:], AF.Copy)
                        nc.sync.dma_start(
                            out_h[s, 2 * je : 2 * je + 2, :, :].rearrange(
                                "r (p q) f -> p r (q f)", p=128
                            ),
                            onat[:],
                        )

            _mstack.close()

    nc.compile()
    return nc


def _get_nc(bias_nonzero=False):
    key = bool(bias_nonzero)
    if key not in _CACHE:
        _CACHE[key] = _build_nc(key)
    return _CACHE[key]


def kernel(feature_map, latent, dense_w, dense_b, conv_kernel, bias):
    bias = np.ascontiguousarray(bias, dtype=np.float32)
    nc = _get_nc(bool(np.any(bias)))
    feature_map = np.ascontiguousarray(feature_map, dtype=np.float32)
    latent = np.ascontiguousarray(latent, dtype=np.float32)
    ident = np.eye(128, dtype=np.float32)
    in_maps = []
    for i in range(NCORES):
        in_maps.append(
            {
                "x": np.ascontiguousarray(feature_map[BS * i : BS * (i + 1)]),
                "lat": np.ascontiguousarray(latent[BS * i : BS * (i + 1)]),
                "dw": np.ascontiguousarray(dense_w, dtype=np.float32),
                "db": np.ascontiguousarray(dense_b, dtype=np.float32),
                "ck": np.ascontiguousarray(conv_kernel, dtype=np.float32),
                "bi": bias,
                "ident": ident,
            }
        )
    res = run_bass_kernel_spmd(nc, in_maps, core_ids=list(range(NCORES)))
    outs = [res.results[i]["out"] for i in range(NCORES)]
    full = np.concatenate(outs, axis=0)
    if getattr(res, "exec_time_ns", None):
        kernel.last_exec_time_ns = res.exec_time_ns
    return full


kernel.last_exec_time_ns = None


# revision 11
# speedup vs baseline: 1.5252x; 1.0002x over previous
"""Demodulated 3x3 convolution Trainium2 kernel — data-stationary odd-pair
scheme.

Data-parallel over batch: 16 samples -> 8 cores x 2 samples.

Input path: HBM NHWC rows are fetched in 8-row chunks with partition =
w-PAIR (512B contiguous descriptors), Pool/SWDGE DMA casting fp32->bf16
in flight.  PE-transpose turns each row into a channel-major slab
[(q,c) 128 parts, 128 wpair cols]; DVE/ACT copies place it into a
non-circular 258-slot ring with ODD-PAIR lanes: slot sl holds image row
sl-1 as ring[(d,c), 129*sl + u] = x[row, 2u+d-1, c] (parts 0:64 = d=1
lane at col offset 0, parts 64:128 = d=0 lane at offset +1; w=-1/256
halo cols and slots 0/257 are statically zero).

Conv (data-stationary): for output row m, wpair j, out[j, (qo,f)] is
accumulated by 6 matmuls (3 ky-groups g x 2 shifts v) with lhsT =
ring[:, 129*(m+g)+v : +128] (the DATA is stationary; Ldweights are free
in the cost model) and rhs = per-sample weight tile wq[s,g,v]
[(d,c), (qo,f)] holding styled+demodulated tap W[g, kx], kx = 2v+d-qo
when 0<=kx<=2 (3 of 4 blocks per tile; 18 blocks over 6 tiles = the
provable minimum for this tap/domino structure).  N=128 per matmul,
PE/iter = 12 matmuls + 2 transposes = 1792 cyc.

Demodulation is folded into the weights: wq = W * const_k * style[c] *
demod[f] built per-sample on DVE (demod[f] broadcast across partitions
via a PE transpose of a column-replicated rsqrt).  The epilogue is a
single ACT copy Q'[j, 2, (qo,f)] PSUM fp32 -> SBUF, and the SP output
DMA writes 512B descriptors.  Conv bias is zero for this problem; a
nonzero bias selects a variant adding a K=1 matmul per row.

Per-iteration budget (cost model): PE 747ns, ACT ~690, DVE ~583,
DMA engines ~728, HWDGE 625, Pool ~335, SP ~590.
"""

import math
import os
import sys

os.environ["BASS_NEVER_TRACE"] = "1"

for _p in ("/opt/trn_rl_repo",):
    if _p not in sys.path:
        sys.path.insert(0, _p)

import numpy as np

import concourse.bass as bass
import concourse.bacc as bacc
import concourse.mybir as mybir
import concourse.tile as tile
from concourse.bass_utils import run_bass_kernel_spmd

B, H, W, CIN = 16, 256, 256, 64
LATENT, F, KK = 512, 64, 3
NCORES = 8
BS = B // NCORES  # samples per core
SLOT = 129  # ring cols per slot (u = 0..128)
NSLOT = H + 2  # slots 0..257; slot sl holds image row sl-1
RW = SLOT * NSLOT

F32 = mybir.dt.float32
BF16 = mybir.dt.bfloat16
AF = mybir.ActivationFunctionType
ALU = mybir.AluOpType

_CACHE = {}


def _build_nc(bias_nonzero):
    nc = bacc.Bacc("TRN2", target_bir_lowering=False, debug=False)

    x_h = nc.dram_tensor("x", [BS, H, W, CIN], F32, kind="ExternalInput")
    lat_h = nc.dram_tensor("lat", [BS, LATENT], F32, kind="ExternalInput")
    dw_h = nc.dram_tensor("dw", [LATENT, CIN], F32, kind="ExternalInput")
    db_h = nc.dram_tensor("db", [CIN], F32, kind="ExternalInput")
    ck_h = nc.dram_tensor("ck", [KK, KK, CIN, F], F32, kind="ExternalInput")
    bi_h = nc.dram_tensor("bi", [F], F32, kind="ExternalInput")
    id_h = nc.dram_tensor("ident", [128, 128], F32, kind="ExternalInput")
    out_h = nc.dram_tensor("out", [BS, H, W, F], F32, kind="ExternalOutput")

    const_k = math.sqrt(2.0) / math.sqrt(KK * KK * CIN)
    inv_sqrt_lat = 1.0 / math.sqrt(LATENT)

    with tile.TileContext(nc) as tc:
        with (
            tc.tile_pool(name="const", bufs=1) as cpool,
            tc.tile_pool(name="wpool", bufs=1) as wpool,
        ):
            # --- loads: chunk0+ksty0 on Pool; ident/lat/dw on SP ---
            identD = cpool.tile([128, 128], F32)
            nc.sync.dma_start(identD[:], id_h[:])
            latR = cpool.tile([BS, LATENT], F32)
            nc.sync.dma_start(latR[:], lat_h[:])
            dwD = cpool.tile([128, 4, CIN], F32)
            nc.sync.dma_start(
                dwD[:], dw_h[:].rearrange("(j p) f -> p j f", p=128)
            )
            ksty0 = cpool.tile([64, 9, F], F32)

            # DVE: matmul operand copies first so style matmuls start early
            dwT = cpool.tile([128, 4, CIN], F32)
            nc.vector.tensor_copy(dwT[:], dwD[:])
            identB = cpool.tile([128, 128], BF16)
            nc.vector.tensor_copy(identB[:], identD[:])
            ones64 = cpool.tile([64, 1], F32)
            nc.vector.memset(ones64[:], 1.0)
            zero64 = cpool.tile([64, 1], F32)
            nc.vector.memset(zero64[:], 0.0)
            onesT64 = cpool.tile([64, 64], F32)
            nc.vector.memset(onesT64[:], 1.0)
            epsT = cpool.tile([64, F], F32)
            nc.vector.memset(epsT[:], 1e-8 / 64.0)

            ring = cpool.tile([128, RW], BF16)

            wq = {}  # (s, g, v) -> [128,128] bf16 lhs... rhs weight tile
            for _s in range(BS):
                for _g in range(3):
                    for _v in range(2):
                        wt = wpool.tile(
                            [128, 128], BF16, tag=f"wq{_s}_{_g}_{_v}"
                        )
                        wq[(_s, _g, _v)] = wt

            # main-loop pools open BEFORE the prologue pool so the
            # prologue's released zone is never reused
            from contextlib import ExitStack
            _mstack = ExitStack()
            stpool = _mstack.enter_context(tc.tile_pool(name="stage", bufs=4))
            opool = _mstack.enter_context(tc.tile_pool(name="onat", bufs=6))
            tpsum = _mstack.enter_context(
                tc.tile_pool(name="tpsum", bufs=2, space="PSUM"))
            qpsum = _mstack.enter_context(
                tc.tile_pool(name="qpsum", bufs=3, space="PSUM"))

            # Pool queue: ksty0 first (weight build is the prologue critical
            # path), then chunk0, then ring zeroing
            nc.gpsimd.dma_start(
                ksty0[:], ck_h[:].rearrange("ky kx c f -> c (ky kx) f")
            )
            stg0_pre = stpool.tile([128, 8, 128], BF16)
            nc.gpsimd.dma_start(
                stg0_pre[:],
                x_h[0, 0:8, :, :].rearrange("r (p q) c -> p r (q c)", p=128),
            )
            # static ring zeroing: halo cols of every slot + slots 0/257
            ring_v = ring[:].rearrange("p (s c) -> p s c", c=SLOT)
            nc.gpsimd.memset(ring_v[0:64, :, 128:129], 0.0)
            nc.gpsimd.memset(ring_v[64:128, :, 0:1], 0.0)
            nc.gpsimd.memset(ring[:, 0:SLOT], 0.0)
            nc.gpsimd.memset(ring[:, SLOT * (NSLOT - 1) : SLOT * NSLOT], 0.0)

            # ---- prologue: style for both samples ----
            pro = _mstack.enter_context(tc.tile_pool(name="pro", bufs=1))
            prop = _mstack.enter_context(
                tc.tile_pool(name="prop", bufs=1, space="PSUM"))

            psLat = prop.tile([128, 4, BS], F32, tag="p")
            for jj in range(4):
                nc.tensor.transpose(
                    psLat[:, jj, :],
                    latR[:, 128 * jj : 128 * jj + 128],
                    identD[0:BS, 0:BS],
                )
            latT = pro.tile([128, 4, BS], F32)
            nc.vector.tensor_copy(latT[:], psLat[:])
            ps_style = prop.tile([CIN, BS], F32, tag="p")
            for jj in range(4):
                nc.tensor.matmul(
                    ps_style[:],
                    dwT[:, jj, :],
                    latT[:, jj, :],
                    start=(jj == 0),
                    stop=(jj == 3),
                )
            db_t = pro.tile([CIN, 1], F32)
            nc.sync.dma_start(db_t[:], db_h[:].rearrange("(c u) -> c u", u=1))
            db_s = pro.tile([CIN, 1], F32)
            nc.vector.tensor_scalar_mul(db_s[:], db_t[:], const_k)
            styleC = pro.tile([64, BS], F32)
            nc.vector.tensor_scalar(
                styleC[:],
                ps_style[:],
                inv_sqrt_lat * const_k,
                db_s[:],
                op0=ALU.mult,
                op1=ALU.add,
            )

            if bias_nonzero:
                onesRow = pro.tile([1, 128], F32)
                nc.vector.memset(onesRow[:], 1.0)
                biasPair = pro.tile([1, 128], F32)
                nc.sync.dma_start(
                    biasPair[0:1, 0:64],
                    bi_h[:].rearrange("(u c) -> u c", u=1),
                )
                nc.sync.dma_start(
                    biasPair[0:1, 64:128],
                    bi_h[:].rearrange("(u c) -> u c", u=1),
                )

            # demod chains for BOTH samples in the prologue (keeps the mid-
            # loop free of Sqrt act-table reloads): demod[f] = rsqrt(
            # sum_c K2[c,f]*style[c]^2 + eps) with K2 from ksq = ksty0^2.
            ksq = pro.tile([64, 9, F], F32)
            nc.vector.tensor_mul(ksq[:], ksty0[:], ksty0[:])
            style2 = pro.tile([64, BS], F32)
            nc.vector.tensor_mul(style2[:], styleC[:], styleC[:])

            emm = []  # per-sample style*demod outer products [c, f]
            for s in range(BS):
                ps_d = prop.tile([CIN, 1], F32, tag="p")
                for t9 in range(9):
                    nc.tensor.matmul(
                        ps_d[:], ksq[:, t9, :], style2[:, s : s + 1],
                        start=(t9 == 0), stop=False,
                    )
                nc.tensor.matmul(
                    ps_d[:], epsT[:], ones64[:], start=False, stop=True
                )
                rt = pro.tile([64, 1], F32, tag=f"rt{s}")
                nc.scalar.activation(rt[:], ps_d[:], AF.Sqrt, bias=zero64[:])
                dm = pro.tile([64, 1], F32, tag=f"dm{s}")
                nc.vector.reciprocal(dm[:], rt[:])
                # demod[f] broadcast across partitions: repl[c,f]=dm[c],
                # PE transpose -> replT[p,n]=dm[n], fused *style -> M
                repl = pro.tile([64, 64], F32, tag=f"repl{s}")
                nc.scalar.activation(repl[:], onesT64[:], AF.Copy,
                                     scale=dm[:])
                replT = prop.tile([64, 64], F32, tag="p")
                nc.tensor.transpose(replT[:], repl[:], identD[0:64, 0:64])
                mm_t = pro.tile([64, 64], F32, tag=f"M{s}")
                nc.vector.tensor_scalar_mul(
                    mm_t[:], replT[:], styleC[:, s : s + 1]
                )
                emm.append(mm_t)

            def block_ops(s, split_pool):
                """Thunks for sample s's 6 weight tiles: zero-block memsets
                + 18 fused block muls wq_block = ksty0[:,idx,:] * M[c,f]."""
                ops = []
                for g in range(3):
                    ops.append(lambda g=g: nc.vector.memset(
                        wq[(s, g, 0)][64:128, 64:128], 0.0))
                    ops.append(lambda g=g: nc.vector.memset(
                        wq[(s, g, 1)][0:64, 0:64], 0.0))
                blocks = []
                for g in range(3):
                    blocks += [
                        (wq[(s, g, 0)][0:64, 0:64], 3 * g + 1),
                        (wq[(s, g, 0)][0:64, 64:128], 3 * g + 0),
                        (wq[(s, g, 0)][64:128, 0:64], 3 * g + 0),
                        (wq[(s, g, 1)][0:64, 64:128], 3 * g + 2),
                        (wq[(s, g, 1)][64:128, 0:64], 3 * g + 2),
                        (wq[(s, g, 1)][64:128, 64:128], 3 * g + 1),
                    ]
                for i, (dst, idx) in enumerate(blocks):
                    eng = nc.gpsimd if (split_pool and i % 2) else nc.vector
                    ops.append(lambda dst=dst, idx=idx, eng=eng:
                               eng.tensor_mul(dst, ksty0[:, idx, :],
                                              emm[s][:]))
                return ops

            for op in block_ops(0, split_pool=True):
                op()

            # ---- main loop ----
            s1_stg0 = [None]
            for s in range(BS):
                if s == 0:
                    stg0 = stg0_pre
                    shadow = block_ops(1, split_pool=False)
                else:
                    stg0 = s1_stg0[0]
                    shadow = []
                stgs = {0: stg0}
                qtiles = {}
                onat_cur = None

                for t in range(131):
                    if s == 0 and t == 124:
                        # prefetch sample 1's first chunk during s0's tail
                        stg1p = stpool.tile([128, 8, 128], BF16)
                        nc.gpsimd.dma_start(
                            stg1p[:],
                            x_h[1, 0:8, :, :].rearrange(
                                "r (p q) c -> p r (q c)", p=128
                            ),
                        )
                        s1_stg0[0] = stg1p
                    # -- input rows 2t, 2t+1 --
                    if t <= 127:
                        k, r4 = divmod(t, 4)
                        if r4 == 0 and k + 1 <= 31:
                            stgk = stpool.tile([128, 8, 128], BF16)
                            nc.gpsimd.dma_start(
                                stgk[:],
                                x_h[s, 8 * (k + 1) : 8 * (k + 2), :, :]
                                .rearrange("r (p q) c -> p r (q c)", p=128),
                            )
                            stgs[k + 1] = stgk
                            stgs.pop(k - 1, None)
                        chunk = stgs[k]
                        ptA = tpsum.tile([128, 128], BF16, tag="ptA")
                        ptB = tpsum.tile([128, 128], BF16, tag="ptB")
                        nc.tensor.transpose(
                            ptA[:], chunk[:, 2 * r4, :], identB[:]
                        )
                        nc.tensor.transpose(
                            ptB[:], chunk[:, 2 * r4 + 1, :], identB[:]
                        )
                        sc0 = SLOT * (2 * t + 1)
                        sc1 = SLOT * (2 * t + 2)
                        # DVE copies alternate PSUM zones (A,B,A) — reads
                        # of the same PSUM tile back-to-back serialize on
                        # the pipeline tail (+160ns each)
                        nc.vector.tensor_copy(
                            ring[0:64, sc0 : sc0 + 128], ptA[0:64, :]
                        )
                        nc.vector.tensor_copy(
                            ring[0:64, sc1 : sc1 + 128], ptB[0:64, :]
                        )
                        nc.vector.tensor_copy(
                            ring[64:128, sc0 + 1 : sc0 + 129], ptA[64:128, :]
                        )
                        nc.scalar.activation(
                            ring[64:128, sc1 + 1 : sc1 + 129],
                            ptB[64:128, :],
                            AF.Copy,
                        )

                    if s == 0 and shadow and 6 <= t and t % 2 == 0:
                        shadow.pop(0)()

                    # -- matmuls for row-pair jmm = t-2 --
                    jmm = t - 2
                    if 0 <= jmm <= 127:
                        Qp = qpsum.tile([128, 2, 128], F32, tag="q")
                        for r in range(2):
                            m = 2 * jmm + r
                            first = True
                            for g in range(3):
                                base = SLOT * (m + g)
                                for v in range(2):
                                    last = (
                                        g == 2 and v == 1
                                        and not bias_nonzero
                                    )
                                    nc.tensor.matmul(
                                        Qp[:, r, :],
                                        ring[:, base + v : base + v + 128],
                                        wq[(s, g, v)][:],
                                        start=first,
                                        stop=last,
                                        skip_group_check=True,
                                    )
                                    first = False
                            if bias_nonzero:
                                nc.tensor.matmul(
                                    Qp[:, r, :],
                                    onesRow[:],
                                    biasPair[:],
                                    start=False,
                                    stop=True,
                                    skip_group_check=True,
                                )
                        qtiles[jmm] = Qp

                    # -- epilogue for je = t-3 (out-DMA batched 2 iters) --
                    je = t - 3
                    if 0 <= je <= 127:
                        Qe = qtiles.pop(je)
                        if je % 2 == 0:
                            onat_cur = opool.tile([128, 4, 128], F32)
                        h = 2 * (je % 2)
                        nc.scalar.activation(
                            onat_cur[:, h : h + 2, :], Qe[:], AF.Copy
                        )
                        if je % 2 == 1:
                            nc.sync.dma_start(
                                out_h[s, 2 * je - 2 : 2 * je + 2, :, :]
                                .rearrange("r (p q) f -> p r (q f)", p=128),
                                onat_cur[:],
                            )

            _mstack.close()

    nc.compile()
    return nc


def _get_nc(bias_nonzero=False):
    key = bool(bias_nonzero)
    if key not in _CACHE:
        _CACHE[key] = _build_nc(key)
    return _CACHE[key]


def kernel(feature_map, latent, dense_w, dense_b, conv_kernel, bias):
    bias = np.ascontiguousarray(bias, dtype=np.float32)
    nc = _get_nc(bool(np.any(bias)))
    feature_map = np.ascontiguousarray(feature_map, dtype=np.float32)
    latent = np.ascontiguousarray(latent, dtype=np.float32)
    ident = np.eye(128, dtype=np.float32)
    in_maps = []
    for i in range(NCORES):
        in_maps.append(
            {
                "x": np.ascontiguousarray(feature_map[BS * i : BS * (i + 1)]),
                "lat": np.ascontiguousarray(latent[BS * i : BS * (i + 1)]),
                "dw": np.ascontiguousarray(dense_w, dtype=np.float32),
                "db": np.ascontiguousarray(dense_b, dtype=np.float32),
                "ck": np.ascontiguousarray(conv_kernel, dtype=np.float32),
                "bi": bias,
                "ident": ident,
            }
        )
    res = run_bass_kernel_spmd(nc, in_maps, core_ids=list(range(NCORES)))
    outs = [res.results[i]["out"] for i in range(NCORES)]
    full = np.concatenate(outs, axis=0)
    if getattr(res, "exec_time_ns", None):
        kernel.last_exec_time_ns = res.exec_time_ns
    return full


kernel.last_exec_time_ns = None
